# revision 2
# baseline (speedup 1.0000x reference)
"""Vocab-parallel cross-entropy loss kernel for Trainium2 (8 NeuronCores).

loss = sum_t w_t * (logsumexp_v(h_t . W_v) - h_t . W_{label_t}) / (sum_t w_t + 1e-8) / gacc

Sharding: head_weight split along vocab (32000 -> 8 x 4000). Every core computes
logits for all 4096 tokens against its vocab shard in fp8 (e4m3) with
perf_mode=DoubleRow (256-deep contraction per matmul, 2 MACs/cell/cycle),
fp32 PSUM accumulation. Weights are pre-scaled by 16 on the host to dodge the
e4m3 subnormal range; the exp drain descales via the activation input scale.
Per-token exp-sums and picked-logit partials accumulate on the fly; one 32KB
AllReduce combines the softmax normalizer and picked logits; every core
finishes the (identical) scalar loss and core 0's output is returned.

Self-contained: hardcodes shapes from the problem spec; only needs numpy,
ml_dtypes and the concourse (Bass/Tile) stack available in the container.
"""

import os

import numpy as np
import ml_dtypes

os.environ.setdefault("MYCRO_LOCAL_CACHE", "1")

import concourse.bass as bass  # noqa: E402
import concourse.tile as tile  # noqa: E402
from concourse import bacc  # noqa: E402
from concourse import mybir  # noqa: E402
from concourse.bass_utils import run_bass_kernel_spmd  # noqa: E402

F32 = mybir.dt.float32
FP8 = mybir.dt.float8e4
ALU = mybir.AluOpType
ACTF = mybir.ActivationFunctionType
AX = mybir.AxisListType
DR = mybir.MatmulPerfMode.DoubleRow

# Problem shapes (hardcoded per contract).
B, S, H, V = 2, 2048, 4096, 32000
T = B * S                      # 4096 tokens
NCORES = 8
VL = V // NCORES               # 4000 vocab rows per core

P = 128                        # partitions
TT = T // P                    # 32 token tiles
HH = H // P                    # 32 contraction tiles of 128
KH = HH // 2                   # 16 DoubleRow pairs (256-deep contraction each)
CHUNK = 500                    # psum free dim (<=512 f32/bank)
NCH = VL // CHUNK              # 8 chunks per core
NCG = NCH                      # chunk groups (zacc/pacc columns per token tile)
WSCALE = 16.0                  # host premultiplier on W to stay in e4m3 normals

_CACHE = {}


def _build(n_passes=1, single_core=False, hid_bufs=3, ep_bufs=3):
    nc = bacc.Bacc("TRN2", target_bir_lowering=False, debug=False,
                   num_devices=1 if single_core else NCORES)
    hidt = nc.dram_tensor("hidt", [H, T], FP8, kind="ExternalInput")
    wsht = nc.dram_tensor("wsht", [H, VL], FP8, kind="ExternalInput")
    ll = nc.dram_tensor("ll", [P, TT], F32, kind="ExternalInput")
    lw = nc.dram_tensor("lw", [P, TT], F32, kind="ExternalInput")
    loss = nc.dram_tensor("loss", [1, 1], F32, kind="ExternalOutput")

    hid_r = hidt.ap().rearrange("(ho p) t -> p ho t", p=P)    # [128, 32, T]
    wt_r = wsht.ap().rearrange("(ho p) v -> p ho v", p=P)     # [128, 32, VL]

    with tile.TileContext(nc) as tc:
        hp = tc.alloc_tile_pool(name="hp", bufs=hid_bufs)
        ep = tc.alloc_tile_pool(name="ep", bufs=ep_bufs)
        pp = tc.alloc_tile_pool(name="pp", bufs=8, space="PSUM")
        cp = tc.alloc_tile_pool(name="cp", bufs=1)   # persistents/constants
        dp = tc.alloc_tile_pool(name="dp", bufs=1, space="DRAM")

        # ---- constants / persistents ----
        iota = cp.tile([P, CHUNK], F32, tag="iota")
        nc.gpsimd.iota(iota, pattern=[[1, CHUNK]], base=0, channel_multiplier=0,
                       allow_small_or_imprecise_dtypes=True)
        ll_sb = cp.tile([P, TT], F32, tag="ll")
        nc.sync.dma_start(out=ll_sb, in_=ll[:, :])
        lw_sb = cp.tile([P, TT], F32, tag="lw")
        nc.sync.dma_start(out=lw_sb, in_=lw[:, :])
        # shifted labels per chunk-group: col = cg*TT + t  -> ll - 500*cg
        ll_shift = cp.tile([P, NCG * TT], F32, tag="llsh")
        for cg in range(NCG):
            nc.vector.tensor_scalar_add(ll_shift[:, cg * TT:(cg + 1) * TT],
                                        ll_sb, float(-CHUNK * cg))
        # per-(t, chunk-group) partial sums, overwritten once each
        zacc = cp.tile([P, NCG * TT], F32, tag="zacc")
        pacc = cp.tile([P, NCG * TT], F32, tag="pacc")
        # W^T resident in SBUF: [ki, ho, v] fp8, 125KB/partition
        wt = cp.tile([P, HH, VL], FP8, tag="wt")

        # ---- main loop ----
        for _ in range(n_passes):
            for c in range(NCH):
                nc.sync.dma_start(out=wt[:, :, c * CHUNK:(c + 1) * CHUNK],
                                  in_=wt_r[:, :, c * CHUNK:(c + 1) * CHUNK])
            for t in range(TT):
                hid_sb = hp.tile([P, HH, P], FP8, tag="hid")
                nc.sync.dma_start(out=hid_sb, in_=hid_r[:, :, t * P:(t + 1) * P])
                ps = [pp.tile([P, CHUNK], F32, tag="ps", name=f"ps{c}")
                      for c in range(NCH)]
                for c in range(NCH):
                    for j in range(KH):
                        nc.tensor.matmul(
                            ps[c], lhsT=hid_sb[:, 2 * j:2 * j + 2, :],
                            rhs=wt[:, 2 * j:2 * j + 2, c * CHUNK:(c + 1) * CHUNK],
                            perf_mode=DR, start=(j == 0), stop=(j == KH - 1))
                for c in range(NCH):
                    col = c * TT + t
                    esc = ep.tile([P, CHUNK], F32, tag="esc")
                    nc.scalar.activation(esc, ps[c], func=ACTF.Exp,
                                         scale=1.0 / WSCALE,
                                         accum_out=zacc[:, col:col + 1])
                    psc = ep.tile([P, CHUNK], F32, tag="psc")
                    nc.vector.scalar_tensor_tensor(
                        out=psc, in0=iota, scalar=ll_shift[:, col:col + 1],
                        in1=ps[c], op0=ALU.is_equal, op1=ALU.mult,
                        accum_out=pacc[:, col:col + 1])

        # ---- reduce partials and all-reduce ----
        arin = cp.tile([P, 2 * TT], F32, tag="arin")
        nc.vector.reduce_sum(out=arin[:, 0:TT],
                             in_=zacc[:].rearrange("p (c t) -> p t c", c=NCG),
                             axis=AX.X)
        nc.vector.reduce_sum(out=arin[:, TT:2 * TT],
                             in_=pacc[:].rearrange("p (c t) -> p t c", c=NCG),
                             axis=AX.X)
        arsum = cp.tile([P, 2 * TT], F32, tag="arsum")
        if single_core:
            nc.vector.tensor_copy(arsum[:], arin[:])
        else:
            ar_in = dp.tile([P, 2 * TT], F32, tag="ari")
            ar_out = dp.tile([P, 2 * TT], F32, tag="aro")
            nc.gpsimd.dma_start(out=ar_in[:], in_=arin[:, :])
            nc.gpsimd.collective_compute(
                "AllReduce", ALU.add, replica_groups=[list(range(NCORES))],
                ins=[ar_in.opt()], outs=[ar_out.opt()])
            nc.gpsimd.dma_start(out=arsum[:], in_=ar_out[:])

        # ---- finale: loss = sum(w*(log Z - picked/16)) / (sum w + 1e-8) ----
        logz = cp.tile([P, TT], F32, tag="logz")
        nc.scalar.activation(logz, arsum[:, 0:TT], func=ACTF.Ln)
        picked = cp.tile([P, TT], F32, tag="picked")
        nc.vector.tensor_scalar_mul(picked, arsum[:, TT:2 * TT], 1.0 / WSCALE)
        pt = cp.tile([P, TT], F32, tag="pt")
        nc.vector.tensor_tensor(pt, logz, picked, ALU.subtract)
        ptw = cp.tile([P, TT], F32, tag="ptw")
        nc.vector.tensor_tensor(ptw, pt, lw_sb, ALU.mult)
        stats2 = cp.tile([P, 2], F32, tag="stats2")
        nc.vector.reduce_sum(out=stats2[:, 0:1], in_=ptw, axis=AX.X)
        nc.vector.reduce_sum(out=stats2[:, 1:2], in_=lw_sb, axis=AX.X)
        ones = cp.tile([P, 1], F32, tag="ones")
        nc.vector.memset(ones, 1.0)
        ps2 = pp.tile([P, CHUNK], F32, tag="ps", name="ps_fin")
        nc.tensor.matmul(ps2[:1, :2], lhsT=ones[:, 0:1], rhs=stats2[:, 0:2],
                         start=True, stop=True)
        res = cp.tile([1, 4], F32, tag="res")
        nc.vector.tensor_scalar_add(res[:, 1:2], ps2[:1, 1:2], 1e-8)
        nc.vector.reciprocal(res[:, 2:3], res[:, 1:2])
        nc.vector.tensor_tensor(res[:, 0:1], ps2[:1, 0:1], res[:, 2:3], ALU.mult)
        nc.sync.dma_start(out=loss[:, :], in_=res[:, 0:1])

        dp.release(); cp.release(); pp.release(); ep.release(); hp.release()

    nc.compile()
    return nc


def _get_nc():
    if "nc" not in _CACHE:
        _CACHE["nc"] = _build()
    return _CACHE["nc"]


def prep_in_maps(hidden_states, head_weight, labels, loss_weight):
    hid = np.asarray(hidden_states, dtype=np.float32).reshape(T, H)
    W = np.asarray(head_weight, dtype=np.float32)
    lab = np.asarray(labels).reshape(-1).astype(np.int64)
    lwf = np.asarray(loss_weight, dtype=np.float32).reshape(-1)

    # host layout prep: hidden transposed to [H, T] fp8; W scaled x16,
    # transposed per-shard to [H, VL] fp8
    hidt = np.ascontiguousarray(hid.T).astype(ml_dtypes.float8_e4m3)
    lw2 = np.ascontiguousarray(lwf.reshape(TT, P).T)  # [p, t_tile]

    in_maps = []
    for c in range(NCORES):
        llc = lab - c * VL
        llc = np.where((llc >= 0) & (llc < VL), llc, -1).astype(np.float32)
        wsh = (W[c * VL:(c + 1) * VL] * WSCALE).T
        in_maps.append({
            "hidt": hidt,
            "wsht": np.ascontiguousarray(wsh).astype(ml_dtypes.float8_e4m3),
            "ll": np.ascontiguousarray(llc.reshape(TT, P).T),
            "lw": lw2,
        })
    return in_maps


def kernel(hidden_states, head_weight, labels, loss_weight,
           grad_accumulation_steps):
    g = np.asarray(grad_accumulation_steps, dtype=np.float64).reshape(-1)
    gacc = float(g[0]) if g.size else 1.0

    in_maps = prep_in_maps(hidden_states, head_weight, labels, loss_weight)
    nc = _get_nc()
    res = run_bass_kernel_spmd(nc, in_maps, core_ids=list(range(NCORES)),
                               trace=False)
    _CACHE["last_results"] = res
    out = np.float32(res.results[0]["loss"][0, 0] / gacc)
    return np.asarray(out, dtype=np.float32)
